# revision 14
# baseline (speedup 1.0000x reference)
# NodeMamba Trainium2 kernel: edge-MLP + Mamba selective scan over ragged
# graph segments + segment-sum scatter, data-parallel over graphs on 8 cores.
#
# Layout strategy: the whole pipeline runs TRANSPOSED ([feature, edge]) so
# the per-(d,n) selective scan maps onto tensor_tensor_scan along the free
# axis and no on-device transposes are needed until the final scatter.
# Each core gets 2 graphs packed into a fixed 1152-column buffer:
#   [0:4 zeros | graph A (<=576) | gap | col 584: graph B (<=568) | tail]
# State resets at graph starts are implemented by poisoning delta at the
# start columns (exp(A * 1e30) == 0 for A < 0), so one scan per (d-chunk, n)
# covers both graphs. Scatter-add over nodes is a one-hot matmul; per-core
# node partials are summed on the host during unsharding.
import os
import sys

sys.path.insert(0, "/opt/trn_rl_repo")

import numpy as np
import ml_dtypes

BF = ml_dtypes.bfloat16
LW = 1152          # per-core column buffer
COL_A = 4          # graph A start column
COL_B = 584        # graph B start column
CAP_A = COL_B - 4 - COL_A   # 576
CAP_B = LW - COL_B          # 568
E = 512
D = 512
NST = 16
NNODE = 2048
N_CORES = 8
NG = 16
CHUNKS = [(0, 512), (512, 512), (1024, 128)]
LTILES = LW // 128          # 9
# dA_n = exp((n+1) * A_col_n * delta); for tiled A_log, (n+1)=(a+1)+(b+1)
# would allow dA_n = dA_a*dA_b, but we keep it general: per-n exp uses the
# actual A column, and only n>=10 uses the product decomposition, which is
# exact whenever A's columns follow A[:, n] = -(n+1)*A1 (true for the
# standard S4D-real init this module uses).  To stay correct for ARBITRARY
# A_log we detect the structure on the host and disable the decomposition
# if it does not hold.
SQ_PAIRS = {10: (4, 5), 11: (5, 5), 12: (5, 6), 13: (6, 6), 14: (6, 7), 15: (7, 7)}

_CACHE_DIR = "/root/.bass_neff_cache"


def _install_compile_cache():
    """Content-keyed disk cache around the BIR->NEFF compile (walrus is
    slow; identical kernels should not recompile across processes)."""
    from concourse import bass2jax, bass_utils

    if getattr(bass_utils.compile_bir_kernel, "_is_disk_cached", False):
        return
    orig = bass_utils.compile_bir_kernel

    def cached(bir_json, tmpdir, neff_name="file.neff"):
        try:
            os.makedirs(_CACHE_DIR, exist_ok=True)
            import hashlib

            key = hashlib.sha256(bir_json).hexdigest()
            path = os.path.join(_CACHE_DIR, key + ".neff")
            if os.path.exists(path):
                out = os.path.join(tmpdir, neff_name)
                with open(path, "rb") as f, open(out, "wb") as g:
                    g.write(f.read())
                return out
            res = orig(bir_json, tmpdir, neff_name)
            tmp = path + ".tmp%d" % os.getpid()
            with open(res, "rb") as f, open(tmp, "wb") as g:
                g.write(f.read())
            os.replace(tmp, path)
            return res
        except Exception:
            return orig(bir_json, tmpdir, neff_name)

    cached._is_disk_cached = True
    bass_utils.compile_bir_kernel = cached
    bass2jax.compile_bir_kernel = cached


_NC_CACHE = {}


def _build_program(use_sq):
    key = ("nc", use_sq)
    if key in _NC_CACHE:
        return _NC_CACHE[key]
    import concourse.bass as bass
    from concourse import bacc
    import concourse.mybir as mybir
    from concourse.tile import TileContext
    from concourse.masks import make_identity

    f32, bf16, i16 = mybir.dt.float32, mybir.dt.bfloat16, mybir.dt.int16
    AL = mybir.AluOpType
    AF = mybir.ActivationFunctionType

    nc = bacc.Bacc("TRN2", target_bir_lowering=False, debug=False,
                   num_devices=N_CORES)

    catT_d = nc.dram_tensor("catT", [13, 128, LW], bf16, kind="ExternalInput")
    w1T_d = nc.dram_tensor("w1T", [13, 128, E], bf16, kind="ExternalInput")
    inpT_d = nc.dram_tensor("inpT", [4, 128, 2 * D], bf16, kind="ExternalInput")
    xpT_d = nc.dram_tensor("xpT", [4, 128, 96], bf16, kind="ExternalInput")
    dtpT_d = nc.dram_tensor("dtpT", [32, D], bf16, kind="ExternalInput")
    opT_d = nc.dram_tensor("opT", [4, 128, E], bf16, kind="ExternalInput")
    asc_d = nc.dram_tensor("asc", [128, 64], f32, kind="ExternalInput")
    dtb_d = nc.dram_tensor("dtb", [128, 4], f32, kind="ExternalInput")
    cw_d = nc.dram_tensor("cw", [128, 16], f32, kind="ExternalInput")
    cb_d = nc.dram_tensor("cb", [128, 4], f32, kind="ExternalInput")
    dv_d = nc.dram_tensor("dv", [128, 4], f32, kind="ExternalInput")
    idx_d = nc.dram_tensor("idxf", [128, LTILES], f32, kind="ExternalInput")
    out_d = nc.dram_tensor("part", [NNODE, E], f32, kind="ExternalOutput")
    sel_d = nc.dram_tensor("sel", [16, 2048], bf16, kind="ExternalInput")

    with TileContext(nc) as tc:
        with (
            tc.tile_pool(name="wpool", bufs=1) as wpool,
            tc.tile_pool(name="cpool", bufs=1) as cpool,
            tc.tile_pool(name="mid", bufs=1) as mid,
        ):
            # ---- weights / constants (persistent) ----
            w1T = [wpool.tile([128, E], bf16, tag=f"w1T{k}", name=f"w1T{k}") for k in range(13)]
            for k in range(13):
                nc.sync.dma_start(w1T[k][:], w1T_d.ap()[k])
            inpT = [wpool.tile([128, 2 * D], bf16, tag=f"inpT{k}", name=f"inpT{k}") for k in range(4)]
            for k in range(4):
                nc.sync.dma_start(inpT[k][:], inpT_d.ap()[k])
            xpT = [wpool.tile([128, 96], bf16, tag=f"xpT{k}", name=f"xpT{k}") for k in range(4)]
            for k in range(4):
                nc.sync.dma_start(xpT[k][:], xpT_d.ap()[k])
            dtpT = wpool.tile([32, D], bf16)
            nc.sync.dma_start(dtpT[:], dtpT_d.ap())
            opT = [wpool.tile([128, E], bf16, tag=f"opT{k}", name=f"opT{k}") for k in range(4)]
            for k in range(4):
                nc.sync.dma_start(opT[k][:], opT_d.ap()[k])
            asc = cpool.tile([128, 64], f32); nc.sync.dma_start(asc[:], asc_d.ap())
            dtb = cpool.tile([128, 4], f32); nc.sync.dma_start(dtb[:], dtb_d.ap())
            cw = cpool.tile([128, 16], f32); nc.sync.dma_start(cw[:], cw_d.ap())
            cb = cpool.tile([128, 4], f32); nc.sync.dma_start(cb[:], cb_d.ap())
            dv = cpool.tile([128, 4], f32); nc.sync.dma_start(dv[:], dv_d.ap())
            idxf = cpool.tile([128, LTILES], f32); nc.sync.dma_start(idxf[:], idx_d.ap())
            ident = cpool.tile([128, 128], bf16)
            make_identity(nc, ident[:])
            sel = cpool.tile([16, 2048], bf16)
            nc.sync.dma_start(sel[:], sel_d.ap())

            # ---- mid-lived tensors ----
            xc = [mid.tile([128, LW], bf16, tag=f"xc{g}", name=f"xc{g}") for g in range(4)]
            gz = [mid.tile([128, LW], bf16, tag=f"gz{g}", name=f"gz{g}") for g in range(4)]
            ut = [mid.tile([128, LW], bf16, tag=f"ut{g}", name=f"ut{g}") for g in range(4)]
            deltaT = [mid.tile([128, LW], f32, tag=f"deltaT{g}", name=f"deltaT{g}") for g in range(4)]
            yT = [mid.tile([128, LW], bf16, tag=f"yT{g}", name=f"yT{g}") for g in range(4)]
            BT = mid.tile([16, LW], bf16)
            CT = mid.tile([16, LW], bf16)

            with tc.tile_pool(name="early", bufs=1) as early:
                x_in = [early.tile([128, LW], bf16, tag=f"x_in{g}", name=f"x_in{g}") for g in range(4)]
                zt = [early.tile([128, LW], bf16, tag=f"zt{g}", name=f"zt{g}") for g in range(4)]
                with (
                    tc.tile_pool(name="ph1", bufs=1) as ph1,
                    tc.tile_pool(name="scr1", bufs=2) as scr1,
                    tc.tile_pool(name="ps1", bufs=2, space="PSUM") as ps1,
                ):
                    catT = [ph1.tile([128, LW], bf16, tag=f"catT{k}", name=f"catT{k}")
                            for k in range(13)]
                    for k in range(13):
                        nc.sync.dma_start(catT[k][:], catT_d.ap()[k])
                    # phase 1: x = lrelu(cat @ w1.T + bias)  [transposed]
                    xT = [ph1.tile([128, LW], bf16, tag=f"xT{m}", name=f"xT{m}") for m in range(4)]
                    for m in range(4):
                        xraw = scr1.tile([128, LW], bf16, tag="xraw")
                        for (c0, cn) in CHUNKS:
                            ps = ps1.tile([128, cn], f32, tag="ps_x")
                            for k in range(13):
                                nc.tensor.matmul(
                                    ps[:], w1T[k][:, m * 128:(m + 1) * 128],
                                    catT[k][:, c0:c0 + cn],
                                    start=(k == 0), stop=(k == 12))
                            nc.scalar.copy(xraw[:, c0:c0 + cn], ps[:])
                        nc.vector.scalar_tensor_tensor(
                            xT[m][:], xraw[:], 0.01, xraw[:], AL.mult, AL.max)
                    # phase 2: x_in, z = x @ in_proj.T
                    for m in range(8):
                        dst = x_in[m] if m < 4 else zt[m - 4]
                        for (c0, cn) in CHUNKS:
                            ps = ps1.tile([128, cn], f32, tag="ps_xz")
                            for k in range(4):
                                nc.tensor.matmul(
                                    ps[:], inpT[k][:, m * 128:(m + 1) * 128],
                                    xT[k][:, c0:c0 + cn],
                                    start=(k == 0), stop=(k == 3))
                            nc.scalar.copy(dst[:, c0:c0 + cn], ps[:])

                # phase 2b: gz = z * sigmoid(z)
                with tc.tile_pool(name="scr2", bufs=2) as scr2:
                    for g in range(4):
                        sg = scr2.tile([128, LW], bf16, tag="sgz")
                        nc.scalar.activation(sg[:], zt[g][:], AF.Sigmoid)
                        nc.vector.tensor_tensor(out=gz[g][:], in0=zt[g][:],
                                                in1=sg[:], op=AL.mult)
                    # phase 3: depthwise causal conv + silu (zero pads mask)
                    for g in range(4):
                        a = scr2.tile([128, LW], bf16, tag="ca")
                        b = scr2.tile([128, LW], bf16, tag="cb_")
                        nc.vector.tensor_scalar(
                            out=a[:, COL_A:], in0=x_in[g][:, COL_A - 3:LW - 3],
                            scalar1=cw[:, g * 4:g * 4 + 1], scalar2=None,
                            op0=AL.mult)
                        nc.vector.scalar_tensor_tensor(
                            b[:, COL_A:], x_in[g][:, COL_A - 2:LW - 2],
                            cw[:, g * 4 + 1:g * 4 + 2], a[:, COL_A:],
                            AL.mult, AL.add)
                        nc.vector.scalar_tensor_tensor(
                            a[:, COL_A:], x_in[g][:, COL_A - 1:LW - 1],
                            cw[:, g * 4 + 2:g * 4 + 3], b[:, COL_A:],
                            AL.mult, AL.add)
                        t = scr2.tile([128, LW], bf16, tag="ct")
                        nc.vector.scalar_tensor_tensor(
                            t[:, COL_A:], x_in[g][:, COL_A:LW],
                            cw[:, g * 4 + 3:g * 4 + 4], a[:, COL_A:],
                            AL.mult, AL.add)
                        t2 = scr2.tile([128, LW], bf16, tag="ct2")
                        nc.vector.tensor_scalar(
                            out=t2[:, COL_A:], in0=t[:, COL_A:],
                            scalar1=cb[:, g:g + 1], scalar2=None, op0=AL.add)
                        nc.gpsimd.memset(t2[:, 0:COL_A], 0.0)
                        sg = scr2.tile([128, LW], bf16, tag="csg")
                        nc.scalar.activation(sg[:], t2[:], AF.Sigmoid)
                        nc.vector.tensor_tensor(out=xc[g][:], in0=t2[:],
                                                in1=sg[:], op=AL.mult)

            # ---- phase 4: dbc = xc @ x_proj.T -> dt, B, C ----
            with (
                tc.tile_pool(name="ph4", bufs=1) as ph4,
                tc.tile_pool(name="scr4", bufs=2) as scr4,
                tc.tile_pool(name="ps4", bufs=2, space="PSUM") as ps4,
            ):
                dtT = ph4.tile([32, LW], bf16)
                for (c0, cn) in CHUNKS:
                    ps = ps4.tile([96, cn], f32, tag="ps_dbc")
                    for k in range(4):
                        nc.tensor.matmul(ps[:], xpT[k][:], xc[k][:, c0:c0 + cn],
                                         start=(k == 0), stop=(k == 3))
                    nc.scalar.copy(dtT[:, c0:c0 + cn], ps[0:32, :])
                    nc.scalar.copy(BT[:, c0:c0 + cn], ps[32:48, :])
                    nc.scalar.copy(CT[:, c0:c0 + cn], ps[64:80, :])
                # phase 5: delta = ln(1 + exp(dt @ dt_proj.T + bias))
                for g in range(4):
                    e1 = scr4.tile([128, LW], f32, tag="sp_e1")
                    for (c0, cn) in CHUNKS:
                        ps = ps4.tile([128, cn], f32, tag="ps_dt")
                        nc.tensor.matmul(ps[:], dtpT[:, g * 128:(g + 1) * 128],
                                         dtT[:, c0:c0 + cn], start=True, stop=True)
                        nc.scalar.activation(e1[:, c0:c0 + cn], ps[:], AF.Exp,
                                             bias=dtb[:, g:g + 1])
                    nc.scalar.activation(deltaT[g][:], e1[:], AF.Ln, bias=1.0)
                # phase 6: u = delta * xc, then poison reset columns of delta
                for g in range(4):
                    nc.vector.tensor_tensor(out=ut[g][:], in0=deltaT[g][:],
                                            in1=xc[g][:], op=AL.mult)
                for g in range(4):
                    nc.gpsimd.memset(deltaT[g][:, COL_A:COL_A + 1], 1.0e30)
                    nc.gpsimd.memset(deltaT[g][:, COL_B:COL_B + 1], 1.0e30)


            # ---- phase 7: scan machinery ----
            # n-inner over a pair of d-chunks; B/C replicated per n via a
            # 1-row DMA stage + gpsimd partition_broadcast; scans on DVE
            # (the compiler rejects the scan opcode on Pool); dA exps on ACT;
            # C-mult tiles split DVE/GPSIMD for engine balance.
            with (
                tc.tile_pool(name="bc", bufs=3) as bcp,
                tc.tile_pool(name="stg", bufs=3) as stg,
                tc.tile_pool(name="dAp", bufs=3) as dAp,
                tc.tile_pool(name="tr", bufs=3) as tr,
                tc.tile_pool(name="ps7", bufs=1, space="PSUM") as ps7,
            ):
                cnt = 0
                for gpair in ((0, 1), (2, 3)):
                    psy = {g: ps7.tile([128, LW], f32, tag=f"ps_y{g % 2}", name=f"ps_y{g}")
                           for g in gpair}
                    for i, n in enumerate(range(16)):
                        sb = stg.tile([1, LW], bf16, tag="sb", name=f"sb{gpair[0]}_{n}")
                        nc.sync.dma_start(sb[:], BT[n:n + 1, :])
                        Brep = bcp.tile([128, LW], bf16, tag="Brep")
                        nc.gpsimd.partition_broadcast(Brep[:], sb[:])
                        sc = stg.tile([1, LW], bf16, tag="sc", name=f"sc{gpair[0]}_{n}")
                        nc.sync.dma_start(sc[:], CT[n:n + 1, :])
                        Crep = bcp.tile([128, LW], bf16, tag="Crep")
                        nc.gpsimd.partition_broadcast(Crep[:], sc[:])
                        for g in gpair:
                            dA_n = dAp.tile([128, LW], bf16, tag=f"dA{g % 2}", name=f"dA_{g}_{n}")
                            nc.scalar.activation(
                                dA_n[:], deltaT[g][:], AF.Exp,
                                scale=asc[:, g * 16 + n:g * 16 + n + 1])
                            dBx = tr.tile([128, LW], bf16, tag="dBx")
                            if cnt % 16 < 7:
                                nc.gpsimd.tensor_tensor(out=dBx[:], in0=ut[g][:],
                                                        in1=Brep[:], op=AL.mult)
                            else:
                                nc.vector.tensor_tensor(out=dBx[:], in0=ut[g][:],
                                                        in1=Brep[:], op=AL.mult)
                            h = tr.tile([128, LW], bf16, tag="h")
                            nc.vector.tensor_tensor_scan(h[:], dA_n[:], dBx[:],
                                                         0.0, AL.mult, AL.add)
                            ch = tr.tile([128, LW], bf16, tag="ch")
                            if cnt % 16 >= 9:
                                nc.gpsimd.tensor_tensor(out=ch[:], in0=h[:],
                                                        in1=Crep[:], op=AL.mult)
                            else:
                                nc.vector.tensor_tensor(out=ch[:], in0=h[:],
                                                        in1=Crep[:], op=AL.mult)
                            cnt += 1
                            for (c0, cn) in CHUNKS:
                                nc.tensor.matmul(psy[g][:, c0:c0 + cn], ident[:],
                                                 ch[:, c0:c0 + cn],
                                                 start=(i == 0), stop=(i == 15))
                    for g in gpair:
                        ytmp = tr.tile([128, LW], bf16, tag="ytmp")
                        nc.vector.scalar_tensor_tensor(ytmp[:], xc[g][:],
                                                       dv[:, g:g + 1], psy[g][:],
                                                       AL.mult, AL.add)
                        nc.vector.tensor_tensor(out=yT[g][:], in0=ytmp[:],
                                                in1=gz[g][:], op=AL.mult)

            # ---- phase 8: out_proj (transposed) + transpose back ----
            with tc.tile_pool(name="ph8", bufs=1) as ph8:
                y_out = [ph8.tile([128, E], bf16, tag=f"y_out{t}", name=f"y_out{t}")
                         for t in range(LTILES)]
                with tc.tile_pool(name="ps8", bufs=2, space="PSUM") as ps8:
                    outT = [ph8.tile([128, LW], bf16, tag=f"outT{m}", name=f"outT{m}")
                            for m in range(4)]
                    for m in range(4):
                        for (c0, cn) in CHUNKS:
                            ps = ps8.tile([128, cn], f32, tag="ps_op")
                            for k in range(4):
                                nc.tensor.matmul(
                                    ps[:], opT[k][:, m * 128:(m + 1) * 128],
                                    yT[k][:, c0:c0 + cn],
                                    start=(k == 0), stop=(k == 3))
                            nc.scalar.copy(outT[m][:, c0:c0 + cn], ps[:])
                    for t in range(LTILES):
                        for m in range(4):
                            pst = ps8.tile([128, 128], bf16, tag="ps_tr")
                            nc.tensor.transpose(
                                pst[:], outT[m][:, t * 128:(t + 1) * 128],
                                ident[:])
                            if (t + m) % 2 == 0:
                                nc.vector.tensor_copy(
                                    y_out[t][:, m * 128:(m + 1) * 128], pst[:])
                            else:
                                nc.scalar.copy(
                                    y_out[t][:, m * 128:(m + 1) * 128], pst[:])

                # ---- phase 9: scatter-add to nodes via one-hot matmul ----
                iot = ph8.tile([128, NNODE], i16)
                nc.gpsimd.iota(iot[:], pattern=[[1, NNODE]], channel_multiplier=0)
                for half in range(2):
                    with (
                        tc.tile_pool(name="ohp", bufs=1) as ohp,
                        tc.tile_pool(name="psS", bufs=1, space="PSUM") as psS,
                    ):
                        ohs = [ohp.tile([128, NNODE // 2], bf16, tag=f"oh{t}", name=f"oh{t}")
                               for t in range(LTILES)]
                        for t in range(LTILES):
                            nc.vector.tensor_scalar(
                                out=ohs[t][:],
                                in0=iot[:, half * 1024:(half + 1) * 1024],
                                scalar1=idxf[:, t:t + 1], scalar2=None,
                                op0=AL.is_equal)
                        pss = [psS.tile([128, E], f32, tag=f"ps_s{mc}", name=f"ps_s{mc}")
                               for mc in range(8)]
                        for t in range(LTILES):
                            for mc in range(8):
                                nc.tensor.matmul(
                                    pss[mc][:], ohs[t][:, mc * 128:(mc + 1) * 128],
                                    y_out[t][:],
                                    start=(t == 0), stop=(t == LTILES - 1))
                        for mc in range(8):
                            node0 = (half * 8 + mc) * 128
                            ob = ohp.tile([128, E], f32, tag=f"ob{mc}",
                                          name=f"ob{half}_{mc}")
                            if mc % 2 == 0:
                                nc.scalar.copy(ob[:], pss[mc][:])
                            else:
                                nc.vector.tensor_copy(ob[:], pss[mc][:])
                            nc.sync.dma_start(out_d.ap()[node0:node0 + 128, :],
                                              ob[:])

    nc.compile()
    _NC_CACHE[key] = nc
    return nc


def _pack_xproj(xp):
    # xp: [64, 512] rows = [dt(32) | B(16) | C(16)] -> lhsT [512, 96] with
    # B at cols 32:48 and C at cols 64:80 (legal psum start partitions)
    out = np.zeros((512, 96), np.float32)
    out[:, 0:32] = xp[0:32].T
    out[:, 32:48] = xp[32:48].T
    out[:, 64:80] = xp[48:64].T
    return out.reshape(4, 128, 96).astype(BF)


def _prep_inputs(inputs):
    q = np.asarray(inputs["q"], np.float32)
    k_v = np.asarray(inputs["k_v"], np.float32)
    k_e = np.asarray(inputs["k_e"], np.float32)
    index = np.asarray(inputs["index"]).astype(np.int64)
    edge_batch = np.asarray(inputs["edge_batch"]).astype(np.int64)

    sizes = np.bincount(edge_batch, minlength=NG)
    starts = np.concatenate([[0], np.cumsum(sizes)[:-1]])
    order = np.argsort(-sizes, kind="stable")
    A_ids, B_ids = order[:8], order[8:][::-1]
    assert sizes[A_ids].max() <= CAP_A and sizes[B_ids].max() <= CAP_B, (
        sizes[A_ids].max(), sizes[B_ids].max())

    cat = np.concatenate([q, k_v, k_e], axis=1)  # [L, 1536]

    in_maps = []
    for c in range(N_CORES):
        ga, gb = A_ids[c], B_ids[c]
        la, lb = int(sizes[ga]), int(sizes[gb])
        ea = slice(int(starts[ga]), int(starts[ga]) + la)
        eb = slice(int(starts[gb]), int(starts[gb]) + lb)

        catT = np.zeros((13 * 128, LW), np.float32)
        catT[:1536, COL_A:COL_A + la] = cat[ea].T
        catT[:1536, COL_B:COL_B + lb] = cat[eb].T
        catT[1536, COL_A:COL_A + la] = 1.0
        catT[1536, COL_B:COL_B + lb] = 1.0

        idxcol = np.full(LW, NNODE, np.float32)
        idxcol[COL_A:COL_A + la] = index[ea]
        idxcol[COL_B:COL_B + lb] = index[eb]
        idxf = idxcol.reshape(LTILES, 128).T.copy()  # [128, 9]

        in_maps.append({
            "catT": catT.reshape(13, 128, LW).astype(BF),
            "idxf": idxf,
        })

    A = -np.exp(np.asarray(inputs["A_log"], np.float32))  # [512, 16]
    # use_sq: dA_{n} products are exact iff A[:, n] == (n+1) * A[:, 0]
    use_sq = bool(np.allclose(A, A[:, 0:1] * np.arange(1, 17, dtype=np.float32),
                              rtol=1e-6, atol=0.0))

    w1T = np.zeros((13 * 128, E), np.float32)
    w1T[:1536] = np.asarray(inputs["w_weight"], np.float32).T
    w1T[1536] = np.asarray(inputs["w_bias"], np.float32)
    selmat = np.zeros((16, 2048), np.float32)
    for n in range(16):
        selmat[n, n * 128:(n + 1) * 128] = 1.0
    shared = {
        "sel": selmat.astype(BF),
        "w1T": w1T.reshape(13, 128, E).astype(BF),
        "inpT": np.asarray(inputs["in_proj_w"], np.float32).T.reshape(4, 128, 2 * D).astype(BF),
        "xpT": _pack_xproj(np.asarray(inputs["x_proj_w"], np.float32)),
        "dtpT": np.asarray(inputs["dt_proj_w"], np.float32).T.astype(BF),
        "opT": np.asarray(inputs["out_proj_w"], np.float32).T.reshape(4, 128, E).astype(BF),
        "asc": A.reshape(4, 128, NST).transpose(1, 0, 2).reshape(128, 64).copy(),
        "dtb": np.asarray(inputs["dt_proj_b"], np.float32).reshape(4, 128).T.copy(),
        "cw": np.asarray(inputs["conv_w"], np.float32).reshape(4, 128, 4).transpose(1, 0, 2).reshape(128, 16).copy(),
        "cb": np.asarray(inputs["conv_b"], np.float32).reshape(4, 128).T.copy(),
        "dv": np.asarray(inputs["D"], np.float32).reshape(4, 128).T.copy(),
    }
    for m in in_maps:
        m.update(shared)
    return in_maps, use_sq


def kernel(**inputs):
    _install_compile_cache()
    from concourse.bass_utils import run_bass_kernel_spmd

    in_maps, use_sq = _prep_inputs(inputs)
    nc = _build_program(use_sq)
    res = run_bass_kernel_spmd(nc, in_maps, list(range(N_CORES)))
    out = np.zeros((NNODE, E), np.float32)
    for r in res.results:
        out += r["part"]
    return out
